# revision 5
# baseline (speedup 1.0000x reference)
"""DIEN forward kernel for Trainium2 (Bass/Tile), 8-core data-parallel.

kernel(**inputs) takes the FULL unsharded inputs (as produced by
reference.setup_inputs()) and returns the full [2048] float32 output.
It shards the batch 2048 -> 8 x 256 across NeuronCores 0..7, runs one
SPMD Bass program per core (no collectives), and concatenates outputs.

Per-core layout is "feature-major": activations are [feature(<=128
partitions), batch(256 free)] tiles; batch column c = h*128 + p for
half h in {0,1}. The two batch halves run as independent software
pipelines through the sequential GRU / AUGRU scans to keep all engines
busy despite the serial per-step dependency chain.

Numerics: matmuls in bf16 with fp32 PSUM accumulation; gate/state
elementwise in bf16 (DVE/GPSIMD, fp32 internal); sigmoid/tanh/exp/relu
on the scalar engine (fp32 internal). Biases in this model are all
zero; nonzero biases are handled via rank-1 matmuls / activation-bias
and only emitted when the host sees nonzero values.
"""
import numpy as np

B_FULL, L, D = 2048, 200, 128
NCORES = 8
B = B_FULL // NCORES          # 256 per core
HB = 128                      # half-batch
ITEM_D, USER_D, CAT_D, DUR_D = 64, 64, 32, 16
UDENSE, IDENSE = 25, 3
MLP1, MLP2 = 256, 128
N_USERS, N_ITEMS, N_CAT, N_DUR = 100000, 100000, 1000, 10

KG = 8          # history-gather chunk (timesteps per indirect DMA)
XRING = 12      # xT ring depth (steps)
ARING = 12      # attention-broadcast ring depth (steps)
ABC = 4         # steps per attention broadcast DMA

# MLP concat feature groups: (name, col offset in W1, width)
GROUPS = [("user", 0, USER_D), ("item", 64, ITEM_D), ("cat", 128, CAT_D),
          ("dur", 160, DUR_D), ("ud", 176, UDENSE), ("idn", 201, IDENSE),
          ("ev", 204, D)]


def build_dien(L_steps=L, nonzero_bias=False):
    import concourse.bacc as bacc
    import concourse.mybir as mybir
    import concourse.tile as tile
    import concourse.bass as bass
    from concourse.masks import make_identity

    f32, bf16, i32 = mybir.dt.float32, mybir.dt.bfloat16, mybir.dt.int32
    AF = mybir.ActivationFunctionType
    OP = mybir.AluOpType

    nc = bacc.Bacc("TRN2", target_bir_lowering=False)

    # ---- DRAM I/O ----
    seq_d = nc.dram_tensor("history_seq", [B, L_steps], i32, kind="ExternalInput")
    uid_d = nc.dram_tensor("user_id", [B], i32, kind="ExternalInput")
    iid_d = nc.dram_tensor("item_id", [B], i32, kind="ExternalInput")
    cid_d = nc.dram_tensor("item_category", [B], i32, kind="ExternalInput")
    did_d = nc.dram_tensor("item_dur_bkt", [B], i32, kind="ExternalInput")
    ud_d = nc.dram_tensor("user_dense", [B, UDENSE], f32, kind="ExternalInput")
    id_d = nc.dram_tensor("item_dense", [B, IDENSE], f32, kind="ExternalInput")
    utab_d = nc.dram_tensor("user_table", [N_USERS, USER_D], f32, kind="ExternalInput")
    itab_d = nc.dram_tensor("item_table", [N_ITEMS, ITEM_D], f32, kind="ExternalInput")
    ctab_d = nc.dram_tensor("cat_table", [N_CAT, CAT_D], f32, kind="ExternalInput")
    dtab_d = nc.dram_tensor("dur_table", [N_DUR, DUR_D], f32, kind="ExternalInput")
    htab_d = nc.dram_tensor("hist_table", [N_ITEMS + 1, D], f32, kind="ExternalInput")
    tpw_d = nc.dram_tensor("target_proj_W", [D, ITEM_D], f32, kind="ExternalInput")
    gwi_d = nc.dram_tensor("gru_Wi", [3 * D, D], f32, kind="ExternalInput")
    gwh_d = nc.dram_tensor("gru_Wh", [3 * D, D], f32, kind="ExternalInput")
    gbi_d = nc.dram_tensor("gru_bi", [3 * D], f32, kind="ExternalInput")
    gbh_d = nc.dram_tensor("gru_bh", [3 * D], f32, kind="ExternalInput")
    awr_d = nc.dram_tensor("au_Wr", [D, 2 * D], f32, kind="ExternalInput")
    abr_d = nc.dram_tensor("au_br", [D], f32, kind="ExternalInput")
    awu_d = nc.dram_tensor("au_Wu", [D, 2 * D], f32, kind="ExternalInput")
    abu_d = nc.dram_tensor("au_bu", [D], f32, kind="ExternalInput")
    awh_d = nc.dram_tensor("au_Wh", [D, 2 * D], f32, kind="ExternalInput")
    abh_d = nc.dram_tensor("au_bh", [D], f32, kind="ExternalInput")
    w1_d = nc.dram_tensor("mlp_W1", [MLP1, 332], f32, kind="ExternalInput")
    b1_d = nc.dram_tensor("mlp_b1", [MLP1], f32, kind="ExternalInput")
    w2_d = nc.dram_tensor("mlp_W2", [MLP2, MLP1], f32, kind="ExternalInput")
    b2_d = nc.dram_tensor("mlp_b2", [MLP2], f32, kind="ExternalInput")
    w3_d = nc.dram_tensor("mlp_W3", [1, MLP2], f32, kind="ExternalInput")
    b3_d = nc.dram_tensor("mlp_b3", [1], f32, kind="ExternalInput")
    out_d = nc.dram_tensor("out", [B], f32, kind="ExternalOutput")

    attnT_dram = nc.dram_tensor("attnT_scr", [L_steps, B], bf16, kind="Internal")
    xdmy_dram = nc.dram_tensor("xdmy_scr", [1, 64], bf16, kind="Internal")

    ntc = max(1, (L_steps + 127) // 128)  # number of 128-step score chunks

    with tile.TileContext(nc) as tc:
        import contextlib
        ctx = contextlib.ExitStack()
        with ctx:
            P = ctx.enter_context(tc.tile_pool(name="persist", bufs=1))
            WK = ctx.enter_context(tc.tile_pool(name="work", bufs=3))
            ST = ctx.enter_context(tc.tile_pool(name="stage", bufs=3))
            PS = ctx.enter_context(tc.tile_pool(name="psum", bufs=2, space="PSUM"))

            # ======== constants ========
            ident_bf = P.tile([128, 128], bf16)
            make_identity(nc, ident_bf[:])
            ident_f32 = P.tile([128, 128], f32)
            make_identity(nc, ident_f32[:])
            zeros_bf = P.tile([128, B], bf16)
            nc.vector.memset(zeros_bf[:], 0.0)

            # ======== per-batch inputs ========
            seq_sb = P.tile([128, 2, L_steps], i32)
            nc.sync.dma_start(out=seq_sb[:], in_=seq_d.rearrange("(h p) t -> p h t", h=2))
            ids_sb = P.tile([128, 4, 2], i32)
            nc.sync.dma_start(out=ids_sb[:, 0, :], in_=uid_d.rearrange("(h p) -> p h", h=2))
            nc.sync.dma_start(out=ids_sb[:, 1, :], in_=iid_d.rearrange("(h p) -> p h", h=2))
            nc.sync.dma_start(out=ids_sb[:, 2, :], in_=cid_d.rearrange("(h p) -> p h", h=2))
            nc.sync.dma_start(out=ids_sb[:, 3, :], in_=did_d.rearrange("(h p) -> p h", h=2))
            ud_sb = P.tile([128, 2, UDENSE], f32)
            nc.sync.dma_start(out=ud_sb[:], in_=ud_d.rearrange("(h p) d -> p h d", h=2))
            idn_sb = P.tile([128, 2, IDENSE], f32)
            nc.sync.dma_start(out=idn_sb[:], in_=id_d.rearrange("(h p) d -> p h d", h=2))

            # ======== weight prep ========
            def load_T(dst_bf, src_ap, rows, cols, scale=1.0):
                """dst_bf <- bf16(transpose(src_ap[rows, cols])) * scale."""
                stg = WK.tile([128, 128], f32, tag="wstg")
                nc.sync.dma_start(out=stg[:rows, :cols], in_=src_ap)
                pst = PS.tile([128, 256], f32, tag="sc")
                nc.tensor.transpose(pst[:cols, :rows], stg[:rows, :cols],
                                    ident_f32[:rows, :rows])
                nc.scalar.activation(out=dst_bf, in_=pst[:cols, :rows], func=AF.Copy,
                                     scale=float(scale))

            wiT = P.tile([128, 3, 128], bf16)
            whT = P.tile([128, 3, 128], bf16)
            for g in range(3):
                load_T(wiT[:, g, :], gwi_d[g * 128:(g + 1) * 128, :], 128, 128)
                load_T(whT[:, g, :], gwh_d[g * 128:(g + 1) * 128, :], 128, 128)
            auT = P.tile([128, 6, 128], bf16)   # rx, rh, ux, uh, hx, hh
            for gi, wd in enumerate((awr_d, awu_d, awh_d)):
                load_T(auT[:, 2 * gi + 0, :], wd[:, 0:128], 128, 128)
                load_T(auT[:, 2 * gi + 1, :], wd[:, 128:256], 128, 128)
            tpT = P.tile([ITEM_D, 128], bf16)
            load_T(tpT[:], tpw_d[:, :], D, ITEM_D, scale=1.0 / float(np.sqrt(D)))
            # MLP W1 per feature group, per m-chunk
            w1T = {}
            for (gname, off, w) in GROUPS:
                tl = P.tile([w, 2, 128], bf16, tag=f"w1T_{gname}", name=f"w1T_{gname}")
                for m in range(2):
                    load_T(tl[:w, m, :], w1_d[m * 128:(m + 1) * 128, off:off + w], 128, w)
                w1T[gname] = tl
            w2T = P.tile([128, 2, 128], bf16)
            for m in range(2):
                load_T(w2T[:, m, :], w2_d[:, m * 128:(m + 1) * 128], 128, 128)
            w3T = P.tile([128, 1], bf16)
            load_T(w3T[:], w3_d[:, :], 1, 128)

            # biases
            gbi_sb = P.tile([128, 3], f32)
            nc.sync.dma_start(out=gbi_sb[:], in_=gbi_d.rearrange("(g p) -> p g", g=3))
            gbh_sb = P.tile([128, 3], f32)
            nc.sync.dma_start(out=gbh_sb[:], in_=gbh_d.rearrange("(g p) -> p g", g=3))
            grz_bias = P.tile([128, 3], f32)
            nc.vector.tensor_tensor(out=grz_bias[:], in0=gbi_sb[:], in1=gbh_sb[:], op=OP.add)
            aub_sb = P.tile([128, 3], f32)
            nc.sync.dma_start(out=aub_sb[:, 0:1], in_=abr_d.rearrange("(a p) -> p a", a=1))
            nc.sync.dma_start(out=aub_sb[:, 1:2], in_=abu_d.rearrange("(a p) -> p a", a=1))
            nc.sync.dma_start(out=aub_sb[:, 2:3], in_=abh_d.rearrange("(a p) -> p a", a=1))
            b1_sb = P.tile([128, 2], f32)
            nc.sync.dma_start(out=b1_sb[:], in_=b1_d.rearrange("(m p) -> p m", m=2))
            b2_sb = P.tile([128, 1], f32)
            nc.sync.dma_start(out=b2_sb[:], in_=b2_d.rearrange("(a p) -> p a", a=1))
            b3_sb = P.tile([1, 1], f32)
            nc.sync.dma_start(out=b3_sb[:], in_=b3_d.rearrange("(a p) -> p a", a=1))
            if nonzero_bias:
                bsel = P.tile([1, 256], bf16)
                nc.vector.memset(bsel[:], 1.0)
                grz_bias_bf = P.tile([128, 3], bf16)
                nc.vector.tensor_copy(out=grz_bias_bf[:], in_=grz_bias[:])
                gbh_bf = P.tile([128, 3], bf16)
                nc.vector.tensor_copy(out=gbh_bf[:], in_=gbh_sb[:])
                aub_bf = P.tile([128, 3], bf16)
                nc.vector.tensor_copy(out=aub_bf[:], in_=aub_sb[:])

            # ======== small embedding gathers (feature-major group tiles) ========
            grp_sb = {}
            for (gname_, _, w_) in GROUPS:
                if gname_ != "ev":
                    grp_sb[gname_] = P.tile([w_, B], bf16, tag=f"xg_{gname_}",
                                            name=f"xg_{gname_}")

            def gather_T(tab_ap, idx_k, width, dst):
                for h in range(2):
                    g = WK.tile([128, width], f32, tag=f"g{width}", name=f"g{width}_{h}")
                    nc.gpsimd.indirect_dma_start(
                        out=g[:], out_offset=None, in_=tab_ap,
                        in_offset=bass.IndirectOffsetOnAxis(
                            ap=ids_sb[:, idx_k, h:h + 1], axis=0))
                    pst2 = PS.tile([128, 256], f32, tag="sc")
                    nc.tensor.transpose(pst2[:width, :128], g[:], ident_f32[:])
                    nc.scalar.activation(out=dst[:, h * HB:(h + 1) * HB],
                                         in_=pst2[:width, :128], func=AF.Copy)

            gather_T(utab_d[:, :], 0, USER_D, grp_sb["user"])
            gather_T(itab_d[:, :], 1, ITEM_D, grp_sb["item"])
            gather_T(ctab_d[:, :], 2, CAT_D, grp_sb["cat"])
            gather_T(dtab_d[:, :], 3, DUR_D, grp_sb["dur"])
            for h in range(2):
                pst3 = PS.tile([128, 256], f32, tag="sc")
                nc.tensor.transpose(pst3[:UDENSE, :128], ud_sb[:, h, :], ident_f32[:])
                nc.scalar.activation(out=grp_sb["ud"][:, h * HB:(h + 1) * HB],
                                     in_=pst3[:UDENSE, :128], func=AF.Copy)
                pst4 = PS.tile([128, 256], f32, tag="sc")
                nc.tensor.transpose(pst4[:IDENSE, :128], idn_sb[:, h, :], ident_f32[:])
                nc.scalar.activation(out=grp_sb["idn"][:, h * HB:(h + 1) * HB],
                                     in_=pst4[:IDENSE, :128], func=AF.Copy)

            # target^T = (1/sqrt(D)) * W_p @ item_emb^T : [D, B] bf16
            tgt_ps = PS.tile([128, 256], f32, tag="sc")
            nc.tensor.matmul(tgt_ps[:, :], tpT[:], grp_sb["item"][:],
                             start=True, stop=True, skip_group_check=True)
            tgt_bf = P.tile([128, B], bf16)
            nc.vector.tensor_copy(out=tgt_bf[:], in_=tgt_ps[:, :])

            # ======== state buffers ========
            hs = P.tile([128, L_steps, B], bf16)
            xT_ring = P.tile([128, XRING, B], bf16)
            a_ring = P.tile([128, ARING, B], bf16)
            ah = P.tile([128, 2, B], bf16)

            # ======== GRU scan ========
            # x pipeline: per-step [P,1] indirect gathers (fp32) into chunk
            # staging, one SWDGE cast-DMA (fp32->bf16) per KG-step chunk,
            # then per-step xbar DMA transposes into the xT ring.
            NCHUNK = (L_steps + KG - 1) // KG

            def issue_hgather(t):
                c = t // KG
                for h in range(2):
                    g32s = ST.tile([128, KG, D], f32, tag=f"g32s{h}",
                                   name=f"g32s_{c}_{h}", bufs=3)
                    if t % KG == 0:
                        stage32[(c % 3, h)] = g32s
                    nc.gpsimd.indirect_dma_start(
                        out=stage32[(c % 3, h)][:, t % KG, :], out_offset=None,
                        in_=htab_d[:, :],
                        in_offset=bass.IndirectOffsetOnAxis(
                            ap=seq_sb[:, h, t:t + 1], axis=0))

            def issue_cast(c):
                for h in range(2):
                    g16s = ST.tile([128, KG, D], bf16, tag=f"g16s{h}",
                                   name=f"g16s_{c}_{h}", bufs=2)
                    stage16[(c % 2, h)] = g16s
                    nc.gpsimd.dma_start(out=g16s[:], in_=stage32[(c % 3, h)][:])

            def issue_xpose(t):
                c = t // KG
                for h in range(2):
                    nc.sync.dma_start_transpose(
                        out=xT_ring[:, t % XRING, h * HB:(h + 1) * HB],
                        in_=stage16[(c % 2, h)][:, t % KG, :])

            stage32, stage16 = {}, {}
            for t in range(min(2 * KG, L_steps)):
                issue_hgather(t)
            issue_cast(0)
            for t in range(min(6, L_steps)):
                issue_xpose(t)

            for t in range(L_steps):
                if t + 2 * KG < L_steps:
                    issue_hgather(t + 2 * KG)
                if (t + KG) % KG == 0 and t + KG < L_steps:
                    issue_cast((t + KG) // KG)
                if t + 6 < L_steps:
                    issue_xpose(t + 6)

                xT_t = xT_ring[:, t % XRING, :]
                h_prev = zeros_bf[:] if t == 0 else hs[:, t - 1, :]

                rz_ps = PS.tile([128, 512], f32, tag="rz")
                nxh_ps = PS.tile([128, 512], f32, tag="nxh")
                nc.tensor.matmul(rz_ps[:, 0:256], wiT[:, 0, :], xT_t,
                                 start=True, stop=False, skip_group_check=True)
                nc.tensor.matmul(rz_ps[:, 256:512], wiT[:, 1, :], xT_t,
                                 start=True, stop=False, skip_group_check=True)
                nc.tensor.matmul(nxh_ps[:, 0:256], wiT[:, 2, :], xT_t,
                                 start=True, stop=False, skip_group_check=True)
                if nonzero_bias:
                    nc.tensor.matmul(rz_ps[:, 0:256], grz_bias_bf[:, 0:1], bsel[:],
                                     start=False, stop=False, skip_group_check=True)
                    nc.tensor.matmul(rz_ps[:, 256:512], grz_bias_bf[:, 1:2], bsel[:],
                                     start=False, stop=False, skip_group_check=True)
                    nc.tensor.matmul(nxh_ps[:, 256:512], gbh_bf[:, 2:3], bsel[:],
                                     start=True, stop=False, skip_group_check=True)
                for h in range(2):
                    hp = h_prev[:, h * HB:(h + 1) * HB]
                    nc.tensor.matmul(rz_ps[:, h * HB:h * HB + HB], whT[:, 0, :], hp,
                                     start=False, stop=False, skip_group_check=True)
                    nc.tensor.matmul(rz_ps[:, 256 + h * HB:256 + h * HB + HB],
                                     whT[:, 1, :], hp,
                                     start=False, stop=True, skip_group_check=True)
                    nc.tensor.matmul(nxh_ps[:, 256 + h * HB:256 + h * HB + HB],
                                     whT[:, 2, :], hp,
                                     start=(not nonzero_bias), stop=True,
                                     skip_group_check=True)
                rz4 = rz_ps[:].rearrange("p (a b) -> p a b", b=128)
                t_sb = WK.tile([128, B], bf16, tag="t_t")
                for h in range(2):
                    rzsb = WK.tile([128, 256], bf16, tag=f"rz{h}")
                    nc.scalar.activation(out=rzsb[:].rearrange("p (a b) -> p a b", b=128),
                                         in_=rz4[:, h::2, :], func=AF.Sigmoid)
                    nc.vector.tensor_tensor(
                        out=t_sb[:, h * HB:(h + 1) * HB], in0=rzsb[:, 0:128],
                        in1=nxh_ps[:, 256 + h * HB:256 + h * HB + HB], op=OP.mult)
                    nc.tensor.matmul(nxh_ps[:, h * HB:h * HB + HB], ident_bf[:],
                                     t_sb[:, h * HB:(h + 1) * HB],
                                     start=False, stop=True, skip_group_check=True)
                    n_sb = WK.tile([128, 128], bf16, tag=f"n{h}")
                    nc.scalar.activation(out=n_sb[:], in_=nxh_ps[:, h * HB:h * HB + HB],
                                         func=AF.Tanh,
                                         bias=gbi_sb[:, 2:3] if nonzero_bias else 0.0)
                    d_sb = WK.tile([128, 128], bf16, tag=f"d{h}")
                    nc.gpsimd.tensor_tensor(out=d_sb[:],
                                            in0=h_prev[:, h * HB:(h + 1) * HB],
                                            in1=n_sb[:], op=OP.subtract)
                    e_sb = WK.tile([128, 128], bf16, tag=f"e{h}")
                    nc.vector.tensor_tensor(out=e_sb[:], in0=rzsb[:, 128:256],
                                            in1=d_sb[:], op=OP.mult)
                    nc.vector.tensor_tensor(out=hs[:, t, h * HB:(h + 1) * HB],
                                            in0=n_sb[:], in1=e_sb[:], op=OP.add)

            # ======== attention scores (PE; scores^T layout) ========
            tchunks = []
            tc0 = 0
            while tc0 < L_steps:
                tchunks.append((tc0, min(128, L_steps - tc0)))
                tc0 += 128
            sc_sb = P.tile([128, ntc, B], f32)
            for ci, (c0, rows) in enumerate(tchunks):
                sc_ps = PS.tile([128, 256], f32, tag="sc")
                for b in range(B):
                    nc.tensor.matmul(sc_ps[:rows, b:b + 1], hs[:, c0:c0 + rows, b:b + 1],
                                     tgt_bf[:, b:b + 1],
                                     start=True, stop=True, skip_group_check=True)
                nc.vector.tensor_copy(out=sc_sb[:rows, ci, :], in_=sc_ps[:rows, :])

            att_work = []
            for h in range(2):
                scb = P.tile([128, L_steps], f32, tag=f"scb{h}")
                for ci, (c0, rows) in enumerate(tchunks):
                    pst5 = PS.tile([128, 256], f32, tag="sc")
                    nc.tensor.transpose(pst5[:128, :rows],
                                        sc_sb[:rows, ci, h * HB:(h + 1) * HB],
                                        ident_f32[:rows, :rows])
                    nc.vector.tensor_copy(out=scb[:, c0:c0 + rows], in_=pst5[:128, :rows])
                att_work.append(scb)

            # ======== mask + softmax (b-major) -> attnT (t-major, bf16) ========
            attnT_sb = P.tile([128, ntc, B], bf16)
            for h in range(2):
                scb = att_work[h]
                m01 = WK.tile([128, L_steps], f32, tag="m01")
                nc.vector.tensor_scalar(out=m01[:], in0=seq_sb[:, h, :], scalar1=0,
                                        scalar2=None, op0=OP.is_gt)
                sm = WK.tile([128, L_steps], f32, tag="sm")
                nc.vector.scalar_tensor_tensor(out=sm[:], in0=scb[:], scalar=1e9,
                                               in1=m01[:], op0=OP.add, op1=OP.mult)
                rmax = WK.tile([128, 1], f32, tag="rmax")
                nc.vector.tensor_reduce(out=rmax[:], in_=sm[:],
                                        axis=mybir.AxisListType.X, op=OP.max, negate=True)
                ex = WK.tile([128, L_steps], f32, tag="ex")
                nc.scalar.activation(out=ex[:], in_=sm[:], func=AF.Exp, bias=rmax[:])
                rsum = WK.tile([128, 1], f32, tag="rsum")
                nc.vector.tensor_reduce(out=rsum[:], in_=ex[:],
                                        axis=mybir.AxisListType.X, op=OP.add)
                rinv = WK.tile([128, 1], f32, tag="rinv")
                nc.vector.reciprocal(out=rinv[:], in_=rsum[:])
                attn_b = WK.tile([128, L_steps], bf16, tag="attnb")
                nc.vector.tensor_scalar(out=attn_b[:], in0=ex[:], scalar1=rinv[:],
                                        scalar2=None, op0=OP.mult)
                for ci, (c0, rows) in enumerate(tchunks):
                    pst6 = PS.tile([128, 256], bf16, tag="scb16")
                    nc.tensor.transpose(pst6[:rows, :128], attn_b[:, c0:c0 + rows],
                                        ident_bf[:])
                    nc.vector.tensor_copy(out=attnT_sb[:rows, ci, h * HB:(h + 1) * HB],
                                          in_=pst6[:rows, :128])
            for ci, (c0, rows) in enumerate(tchunks):
                nc.sync.dma_start(out=attnT_dram[c0:c0 + rows, :],
                                  in_=attnT_sb[:rows, ci, :])

            # ======== AUGRU scan ========
            def issue_abcast(t0):
                kw = min(ABC, L_steps - t0)
                src = attnT_dram[t0:t0 + kw, :]
                src_b = bass.AP(tensor=src.tensor, offset=src.offset,
                                ap=[[0, 128]] + list(src.ap))
                nc.gpsimd.dma_start(out=a_ring[:, t0 % ARING:t0 % ARING + kw, :],
                                    in_=src_b)

            for t0 in range(0, min(ARING, L_steps), ABC):
                issue_abcast(t0)

            for t in range(L_steps):
                tf = t + ARING // 2
                if tf < L_steps and tf % ABC == 0:
                    issue_abcast(tf)
                x_t = hs[:, t, :]
                h_prev = zeros_bf[:] if t == 0 else ah[:, (t - 1) % 2, :]

                ru_ps = PS.tile([128, 512], f32, tag="rz")
                n_ps = PS.tile([128, 256], f32, tag="nxh")
                nc.tensor.matmul(ru_ps[:, 0:256], auT[:, 0, :], x_t,
                                 start=True, stop=False, skip_group_check=True)
                nc.tensor.matmul(ru_ps[:, 256:512], auT[:, 2, :], x_t,
                                 start=True, stop=False, skip_group_check=True)
                nc.tensor.matmul(n_ps[:, 0:256], auT[:, 4, :], x_t,
                                 start=True, stop=False, skip_group_check=True)
                if nonzero_bias:
                    nc.tensor.matmul(ru_ps[:, 0:256], aub_bf[:, 0:1], bsel[:],
                                     start=False, stop=False, skip_group_check=True)
                    nc.tensor.matmul(ru_ps[:, 256:512], aub_bf[:, 1:2], bsel[:],
                                     start=False, stop=False, skip_group_check=True)
                for h in range(2):
                    hp = h_prev[:, h * HB:(h + 1) * HB]
                    nc.tensor.matmul(ru_ps[:, h * HB:h * HB + HB], auT[:, 1, :], hp,
                                     start=False, stop=False, skip_group_check=True)
                    nc.tensor.matmul(ru_ps[:, 256 + h * HB:256 + h * HB + HB],
                                     auT[:, 3, :], hp,
                                     start=False, stop=True, skip_group_check=True)
                ru4 = ru_ps[:].rearrange("p (a b) -> p a b", b=128)
                for h in range(2):
                    hp = h_prev[:, h * HB:(h + 1) * HB]
                    rusb = WK.tile([128, 256], bf16, tag=f"ru{h}")
                    nc.scalar.activation(out=rusb[:].rearrange("p (a b) -> p a b", b=128),
                                         in_=ru4[:, h::2, :], func=AF.Sigmoid)
                    rh_sb = WK.tile([128, 128], bf16, tag=f"rh{h}")
                    nc.vector.tensor_tensor(out=rh_sb[:], in0=rusb[:, 0:128], in1=hp,
                                            op=OP.mult)
                    nc.tensor.matmul(n_ps[:, h * HB:h * HB + HB], auT[:, 5, :], rh_sb[:],
                                     start=False, stop=True, skip_group_check=True)
                    n2_sb = WK.tile([128, 128], bf16, tag=f"n2{h}")
                    nc.scalar.activation(out=n2_sb[:], in_=n_ps[:, h * HB:h * HB + HB],
                                         func=AF.Tanh,
                                         bias=aub_sb[:, 2:3] if nonzero_bias else 0.0)
                    w_sb = WK.tile([128, 128], bf16, tag=f"w{h}")
                    nc.vector.tensor_tensor(out=w_sb[:], in0=rusb[:, 128:256],
                                            in1=a_ring[:, t % ARING, h * HB:(h + 1) * HB],
                                            op=OP.mult)
                    d2_sb = WK.tile([128, 128], bf16, tag=f"d2{h}")
                    nc.gpsimd.tensor_tensor(out=d2_sb[:], in0=n2_sb[:], in1=hp,
                                            op=OP.subtract)
                    e2_sb = WK.tile([128, 128], bf16, tag=f"e2{h}")
                    nc.vector.tensor_tensor(out=e2_sb[:], in0=w_sb[:], in1=d2_sb[:],
                                            op=OP.mult)
                    nc.vector.tensor_tensor(out=ah[:, t % 2, h * HB:(h + 1) * HB],
                                            in0=hp, in1=e2_sb[:], op=OP.add)

            evolved = ah[:, (L_steps - 1) % 2, :]
            grp_rhs = dict(grp_sb)
            grp_rhs["ev"] = evolved

            # ======== MLP head ========
            h1_sb = P.tile([128, 2, B], bf16)
            for m in range(2):
                h1_ps = PS.tile([128, 256], f32, tag="nxh")
                for gi, (gname, off, w) in enumerate(GROUPS):
                    nc.tensor.matmul(h1_ps[:, :], w1T[gname][:w, m, :],
                                     grp_rhs[gname][:] if gname == "ev" else grp_rhs[gname][:w, :],
                                     start=(gi == 0), stop=(gi == len(GROUPS) - 1),
                                     skip_group_check=True)
                nc.scalar.activation(out=h1_sb[:, m, :], in_=h1_ps[:, :], func=AF.Relu,
                                     bias=b1_sb[:, m:m + 1])
            h2_ps = PS.tile([128, 256], f32, tag="rz")
            nc.tensor.matmul(h2_ps[:, :], w2T[:, 0, :], h1_sb[:, 0, :],
                             start=True, stop=False, skip_group_check=True)
            nc.tensor.matmul(h2_ps[:, :], w2T[:, 1, :], h1_sb[:, 1, :],
                             start=False, stop=True, skip_group_check=True)
            h2_sb = P.tile([128, B], bf16)
            nc.scalar.activation(out=h2_sb[:], in_=h2_ps[:, :], func=AF.Relu,
                                 bias=b2_sb[:])
            lg_ps = PS.tile([1, 256], f32, tag="sc")
            nc.tensor.matmul(lg_ps[:, :], w3T[:], h2_sb[:],
                             start=True, stop=True, skip_group_check=True)
            out_sb = P.tile([1, B], f32)
            nc.scalar.activation(out=out_sb[:], in_=lg_ps[:, :], func=AF.Sigmoid,
                                 bias=b3_sb[0:1, :])
            nc.sync.dma_start(out=out_d.rearrange("(a b) -> a b", a=1), in_=out_sb[:])

    nc.finalize()
    return nc


_NC_CACHE = {}


def kernel(**inputs):
    from concourse import bass_utils

    inputs = {k: np.asarray(v) for k, v in inputs.items()}
    L_steps = inputs["history_seq"].shape[1]
    bias_names = ["gru_bi", "gru_bh", "au_br", "au_bu", "au_bh"]
    nonzero_bias = any(np.any(inputs[k]) for k in bias_names)

    key = (L_steps, nonzero_bias)
    if key not in _NC_CACHE:
        _NC_CACHE[key] = build_dien(L_steps, nonzero_bias)
    nc = _NC_CACHE[key]

    per_b = ["user_id", "item_id", "item_category", "item_dur_bkt",
             "history_seq", "user_dense", "item_dense"]
    shared = {k: np.ascontiguousarray(v) for k, v in inputs.items() if k not in per_b}
    in_maps = []
    for c in range(NCORES):
        m = dict(shared)
        for k in per_b:
            m[k] = np.ascontiguousarray(inputs[k][c * B:(c + 1) * B])
        in_maps.append(m)

    res = bass_utils.run_bass_kernel_spmd(nc, in_maps, core_ids=list(range(NCORES)))
    out = np.concatenate([res.results[c]["out"] for c in range(NCORES)])
    return out.astype(np.float32)


if __name__ == "__main__":
    import os
    import importlib.util
    spec = importlib.util.spec_from_file_location("reference", "/root/problem/reference.py")
    ref = importlib.util.module_from_spec(spec)
    spec.loader.exec_module(ref)
    ins = {k: np.asarray(v) for k, v in ref.setup_inputs().items()}
    Lt = int(os.environ.get("DIEN_L", "8"))
    if Lt < L:
        ins["history_seq"] = np.ascontiguousarray(ins["history_seq"][:, :Lt])
    import jax.numpy as jnp
    exp = np.asarray(ref.reference(**{k: jnp.asarray(v) for k, v in ins.items()}))
    got = kernel(**ins)
    err = np.abs(got - exp)
    rel = err / np.maximum(np.abs(exp), 1e-6)
    print(f"L={Lt} max_abs={err.max():.3e} max_rel={rel.max():.3e} mean_rel={rel.mean():.3e}")


# revision 13
# speedup vs baseline: 8710.3855x; 8710.3855x over previous
"""DIEN forward kernel for Trainium2 (Bass/Tile), 8-core data-parallel.

kernel(**inputs) takes the FULL unsharded inputs (as produced by
reference.setup_inputs()) and returns the full [2048] float32 output.
It shards the batch 2048 -> 8 x 256 across NeuronCores 0..7, runs one
SPMD Bass program per core (no collectives), and concatenates outputs.

Per-core layout is "feature-major": activations are [feature(<=128
partitions), batch(256 free)] tiles; batch column c = h*128 + p for
half h in {0,1}. The two batch halves run as independent software
pipelines through the sequential GRU / AUGRU scans to keep all engines
busy despite the serial per-step dependency chain.

Numerics: matmuls in bf16 with fp32 PSUM accumulation; gate/state
elementwise in bf16 (DVE/GPSIMD, fp32 internal); sigmoid/tanh/exp/relu
on the scalar engine (fp32 internal). Biases in this model are all
zero; nonzero biases are handled via rank-1 matmuls / activation-bias
and only emitted when the host sees nonzero values.
"""
import numpy as np

B_FULL, L, D = 2048, 200, 128
NCORES = 8
B = B_FULL // NCORES          # 256 per core
HB = 128                      # half-batch
ITEM_D, USER_D, CAT_D, DUR_D = 64, 64, 32, 16
UDENSE, IDENSE = 25, 3
MLP1, MLP2 = 256, 128
N_USERS, N_ITEMS, N_CAT, N_DUR = 100000, 100000, 1000, 10

KG = 8          # history-gather chunk (timesteps per indirect DMA)
XRING = 12      # xT ring depth (steps)
ARING = 12      # attention-broadcast ring depth (steps)
ABC = 4         # steps per attention broadcast DMA

# MLP concat feature groups: (name, col offset in W1, width)
GROUPS = [("user", 0, USER_D), ("item", 64, ITEM_D), ("cat", 128, CAT_D),
          ("dur", 160, DUR_D), ("ud", 176, UDENSE), ("idn", 201, IDENSE),
          ("ev", 204, D)]


def build_dien(L_steps=L, nonzero_bias=False):
    import concourse.bacc as bacc
    import concourse.mybir as mybir
    import concourse.tile as tile
    import concourse.bass as bass
    from concourse.masks import make_identity

    f32, bf16, i32 = mybir.dt.float32, mybir.dt.bfloat16, mybir.dt.int32
    AF = mybir.ActivationFunctionType
    OP = mybir.AluOpType

    nc = bacc.Bacc("TRN2", target_bir_lowering=False)

    # ---- DRAM I/O ----
    seq_d = nc.dram_tensor("history_seq", [B, L_steps], i32, kind="ExternalInput")
    uid_d = nc.dram_tensor("user_id", [B], i32, kind="ExternalInput")
    iid_d = nc.dram_tensor("item_id", [B], i32, kind="ExternalInput")
    cid_d = nc.dram_tensor("item_category", [B], i32, kind="ExternalInput")
    did_d = nc.dram_tensor("item_dur_bkt", [B], i32, kind="ExternalInput")
    ud_d = nc.dram_tensor("user_dense", [B, UDENSE], f32, kind="ExternalInput")
    id_d = nc.dram_tensor("item_dense", [B, IDENSE], f32, kind="ExternalInput")
    utab_d = nc.dram_tensor("user_table", [N_USERS, USER_D], f32, kind="ExternalInput")
    itab_d = nc.dram_tensor("item_table", [N_ITEMS, ITEM_D], f32, kind="ExternalInput")
    ctab_d = nc.dram_tensor("cat_table", [N_CAT, CAT_D], f32, kind="ExternalInput")
    dtab_d = nc.dram_tensor("dur_table", [N_DUR, DUR_D], f32, kind="ExternalInput")
    htab_d = nc.dram_tensor("hist_table", [N_ITEMS + 1, D], f32, kind="ExternalInput")
    tpw_d = nc.dram_tensor("target_proj_W", [D, ITEM_D], f32, kind="ExternalInput")
    gwi_d = nc.dram_tensor("gru_Wi", [3 * D, D], f32, kind="ExternalInput")
    gwh_d = nc.dram_tensor("gru_Wh", [3 * D, D], f32, kind="ExternalInput")
    gbi_d = nc.dram_tensor("gru_bi", [3 * D], f32, kind="ExternalInput")
    gbh_d = nc.dram_tensor("gru_bh", [3 * D], f32, kind="ExternalInput")
    awr_d = nc.dram_tensor("au_Wr", [D, 2 * D], f32, kind="ExternalInput")
    abr_d = nc.dram_tensor("au_br", [D], f32, kind="ExternalInput")
    awu_d = nc.dram_tensor("au_Wu", [D, 2 * D], f32, kind="ExternalInput")
    abu_d = nc.dram_tensor("au_bu", [D], f32, kind="ExternalInput")
    awh_d = nc.dram_tensor("au_Wh", [D, 2 * D], f32, kind="ExternalInput")
    abh_d = nc.dram_tensor("au_bh", [D], f32, kind="ExternalInput")
    w1_d = nc.dram_tensor("mlp_W1", [MLP1, 332], f32, kind="ExternalInput")
    b1_d = nc.dram_tensor("mlp_b1", [MLP1], f32, kind="ExternalInput")
    w2_d = nc.dram_tensor("mlp_W2", [MLP2, MLP1], f32, kind="ExternalInput")
    b2_d = nc.dram_tensor("mlp_b2", [MLP2], f32, kind="ExternalInput")
    w3_d = nc.dram_tensor("mlp_W3", [1, MLP2], f32, kind="ExternalInput")
    b3_d = nc.dram_tensor("mlp_b3", [1], f32, kind="ExternalInput")
    out_d = nc.dram_tensor("out", [B], f32, kind="ExternalOutput")

    attnT_dram = nc.dram_tensor("attnT_scr", [L_steps, B], bf16, kind="Internal")
    xdmy_dram = nc.dram_tensor("xdmy_scr", [1, 64], bf16, kind="Internal")

    # score chunks: <=128-step blocks; keep a small tail chunk so most of
    # the score matmuls overlap the GRU scan instead of serializing after it
    tchunks = []
    tc0 = 0
    while tc0 < L_steps:
        w = min(int(__import__('os').environ.get('DIEN_CHUNKW', '128')), L_steps - tc0)
        if L_steps - tc0 - w > 0 and L_steps - tc0 - w < 32:
            w = L_steps - tc0 - 16
        tchunks.append((tc0, w))
        tc0 += w
    ntc = len(tchunks)

    with tile.TileContext(nc) as tc:
        import contextlib
        ctx = contextlib.ExitStack()
        with ctx:
            P = ctx.enter_context(tc.tile_pool(name="persist", bufs=1))
            WK = ctx.enter_context(tc.tile_pool(name="work", bufs=4))
            ST = ctx.enter_context(tc.tile_pool(name="stage", bufs=3))
            PS = ctx.enter_context(tc.tile_pool(name="psum", bufs=2, space="PSUM"))

            # ======== constants ========
            ident_bf = P.tile([128, 128], bf16)
            make_identity(nc, ident_bf[:])
            ident_f32 = P.tile([128, 128], f32)
            make_identity(nc, ident_f32[:])
            zeros_bf = P.tile([128, B], bf16)
            nc.vector.memset(zeros_bf[:], 0.0)

            # ======== per-batch inputs ========
            seq_sb = P.tile([128, 2, L_steps], i32)
            nc.sync.dma_start(out=seq_sb[:], in_=seq_d.rearrange("(h p) t -> p h t", h=2))
            ids_sb = P.tile([128, 4, 2], i32)
            nc.sync.dma_start(out=ids_sb[:, 0, :], in_=uid_d.rearrange("(h p) -> p h", h=2))
            nc.sync.dma_start(out=ids_sb[:, 1, :], in_=iid_d.rearrange("(h p) -> p h", h=2))
            nc.sync.dma_start(out=ids_sb[:, 2, :], in_=cid_d.rearrange("(h p) -> p h", h=2))
            nc.sync.dma_start(out=ids_sb[:, 3, :], in_=did_d.rearrange("(h p) -> p h", h=2))
            ud_sb = P.tile([128, 2, UDENSE], f32)
            nc.sync.dma_start(out=ud_sb[:], in_=ud_d.rearrange("(h p) d -> p h d", h=2))
            idn_sb = P.tile([128, 2, IDENSE], f32)
            nc.sync.dma_start(out=idn_sb[:], in_=id_d.rearrange("(h p) d -> p h d", h=2))

            # ======== weight prep ========
            def load_T(dst_bf, src_ap, rows, cols, scale=1.0):
                """dst_bf <- bf16(transpose(src_ap[rows, cols])) * scale."""
                stg = WK.tile([128, 128], f32, tag="wstg")
                nc.sync.dma_start(out=stg[:rows, :cols], in_=src_ap)
                pst = PS.tile([128, 256], f32, tag="sc")
                nc.tensor.transpose(pst[:cols, :rows], stg[:rows, :cols],
                                    ident_f32[:rows, :rows])
                nc.scalar.activation(out=dst_bf, in_=pst[:cols, :rows], func=AF.Copy,
                                     scale=float(scale))

            wiT = P.tile([128, 3, 128], bf16)
            whT = P.tile([128, 3, 128], bf16)
            for g in range(3):
                load_T(wiT[:, g, :], gwi_d[g * 128:(g + 1) * 128, :], 128, 128)
                load_T(whT[:, g, :], gwh_d[g * 128:(g + 1) * 128, :], 128, 128)
            auT = P.tile([128, 6, 128], bf16)   # rx, rh, ux, uh, hx, hh
            for gi, wd in enumerate((awr_d, awu_d, awh_d)):
                load_T(auT[:, 2 * gi + 0, :], wd[:, 0:128], 128, 128)
                load_T(auT[:, 2 * gi + 1, :], wd[:, 128:256], 128, 128)
            tpT = P.tile([ITEM_D, 128], bf16)
            load_T(tpT[:], tpw_d[:, :], D, ITEM_D, scale=1.0 / float(np.sqrt(D)))
            # MLP W1 per feature group, per m-chunk
            w1T = {}
            for (gname, off, w) in GROUPS:
                tl = P.tile([w, 2, 128], bf16, tag=f"w1T_{gname}", name=f"w1T_{gname}")
                for m in range(2):
                    load_T(tl[:w, m, :], w1_d[m * 128:(m + 1) * 128, off:off + w], 128, w)
                w1T[gname] = tl
            w2T = P.tile([128, 2, 128], bf16)
            for m in range(2):
                load_T(w2T[:, m, :], w2_d[:, m * 128:(m + 1) * 128], 128, 128)
            w3T = P.tile([128, 1], bf16)
            load_T(w3T[:], w3_d[:, :], 1, 128)

            # biases
            gbi_sb = P.tile([128, 3], f32)
            nc.sync.dma_start(out=gbi_sb[:], in_=gbi_d.rearrange("(g p) -> p g", g=3))
            gbh_sb = P.tile([128, 3], f32)
            nc.sync.dma_start(out=gbh_sb[:], in_=gbh_d.rearrange("(g p) -> p g", g=3))
            grz_bias = P.tile([128, 3], f32)
            nc.vector.tensor_tensor(out=grz_bias[:], in0=gbi_sb[:], in1=gbh_sb[:], op=OP.add)
            aub_sb = P.tile([128, 3], f32)
            nc.sync.dma_start(out=aub_sb[:, 0:1], in_=abr_d.rearrange("(a p) -> p a", a=1))
            nc.sync.dma_start(out=aub_sb[:, 1:2], in_=abu_d.rearrange("(a p) -> p a", a=1))
            nc.sync.dma_start(out=aub_sb[:, 2:3], in_=abh_d.rearrange("(a p) -> p a", a=1))
            b1_sb = P.tile([128, 2], f32)
            nc.sync.dma_start(out=b1_sb[:], in_=b1_d.rearrange("(m p) -> p m", m=2))
            b2_sb = P.tile([128, 1], f32)
            nc.sync.dma_start(out=b2_sb[:], in_=b2_d.rearrange("(a p) -> p a", a=1))
            b3_sb = P.tile([1, 1], f32)
            nc.sync.dma_start(out=b3_sb[:], in_=b3_d.rearrange("(a p) -> p a", a=1))
            if nonzero_bias:
                bsel = P.tile([1, 256], bf16)
                nc.vector.memset(bsel[:], 1.0)
                grz_bias_bf = P.tile([128, 3], bf16)
                nc.vector.tensor_copy(out=grz_bias_bf[:], in_=grz_bias[:])
                gbh_bf = P.tile([128, 3], bf16)
                nc.vector.tensor_copy(out=gbh_bf[:], in_=gbh_sb[:])
                aub_bf = P.tile([128, 3], bf16)
                nc.vector.tensor_copy(out=aub_bf[:], in_=aub_sb[:])

            # ======== small embedding gathers (feature-major group tiles) ========
            grp_sb = {}
            for (gname_, _, w_) in GROUPS:
                if gname_ != "ev":
                    grp_sb[gname_] = P.tile([w_, B], bf16, tag=f"xg_{gname_}",
                                            name=f"xg_{gname_}")

            def gather_T(tab_ap, idx_k, width, dst):
                for h in range(2):
                    g = WK.tile([128, width], f32, tag=f"g{width}", name=f"g{width}_{h}")
                    nc.gpsimd.indirect_dma_start(
                        out=g[:], out_offset=None, in_=tab_ap,
                        in_offset=bass.IndirectOffsetOnAxis(
                            ap=ids_sb[:, idx_k, h:h + 1], axis=0))
                    pst2 = PS.tile([128, 256], f32, tag="sc")
                    nc.tensor.transpose(pst2[:width, :128], g[:], ident_f32[:])
                    nc.scalar.activation(out=dst[:, h * HB:(h + 1) * HB],
                                         in_=pst2[:width, :128], func=AF.Copy)

            gather_T(utab_d[:, :], 0, USER_D, grp_sb["user"])
            gather_T(itab_d[:, :], 1, ITEM_D, grp_sb["item"])
            gather_T(ctab_d[:, :], 2, CAT_D, grp_sb["cat"])
            gather_T(dtab_d[:, :], 3, DUR_D, grp_sb["dur"])
            for h in range(2):
                pst3 = PS.tile([128, 256], f32, tag="sc")
                nc.tensor.transpose(pst3[:UDENSE, :128], ud_sb[:, h, :], ident_f32[:])
                nc.scalar.activation(out=grp_sb["ud"][:, h * HB:(h + 1) * HB],
                                     in_=pst3[:UDENSE, :128], func=AF.Copy)
                pst4 = PS.tile([128, 256], f32, tag="sc")
                nc.tensor.transpose(pst4[:IDENSE, :128], idn_sb[:, h, :], ident_f32[:])
                nc.scalar.activation(out=grp_sb["idn"][:, h * HB:(h + 1) * HB],
                                     in_=pst4[:IDENSE, :128], func=AF.Copy)

            # target^T = (1/sqrt(D)) * W_p @ item_emb^T : [D, B] bf16
            tgt_ps = PS.tile([128, 256], f32, tag="sc")
            nc.tensor.matmul(tgt_ps[:, :], tpT[:], grp_sb["item"][:],
                             start=True, stop=True, skip_group_check=True)
            tgt_bf = P.tile([128, B], bf16)
            nc.vector.tensor_copy(out=tgt_bf[:], in_=tgt_ps[:, :])

            # ======== state buffers ========
            hs = P.tile([128, L_steps, B], bf16)
            xT_ring = P.tile([128, XRING, B], bf16)
            a_ring = P.tile([128, ARING, B], bf16)
            ah = P.tile([128, 2, B], bf16)

            # ======== GRU scan ========
            # x pipeline: per-step [P,1] indirect gathers (fp32) into chunk
            # staging, one SWDGE cast-DMA (fp32->bf16) per KG-step chunk,
            # then per-step xbar DMA transposes into the xT ring.
            NCHUNK = (L_steps + KG - 1) // KG

            def issue_hgather(t):
                c = t // KG
                for h in range(2):
                    g32s = ST.tile([128, KG, D], f32, tag=f"g32s{h}",
                                   name=f"g32s_{c}_{h}", bufs=2)
                    if t % KG == 0:
                        stage32[(c % 3, h)] = g32s
                    nc.gpsimd.indirect_dma_start(
                        out=stage32[(c % 3, h)][:, t % KG, :], out_offset=None,
                        in_=htab_d[:, :],
                        in_offset=bass.IndirectOffsetOnAxis(
                            ap=seq_sb[:, h, t:t + 1], axis=0))

            def issue_cast(c):
                for h in range(2):
                    g16s = ST.tile([128, KG, D], bf16, tag=f"g16s{h}",
                                   name=f"g16s_{c}_{h}", bufs=2)
                    stage16[(c % 2, h)] = g16s
                    nc.gpsimd.dma_start(out=g16s[:], in_=stage32[(c % 3, h)][:])

            def issue_xpose(t):
                c = t // KG
                for h in range(2):
                    nc.sync.dma_start_transpose(
                        out=xT_ring[:, t % XRING, h * HB:(h + 1) * HB],
                        in_=stage16[(c % 2, h)][:, t % KG, :])

            stage32, stage16 = {}, {}
            for t in range(min(KG, L_steps)):
                issue_hgather(t)
            issue_cast(0)
            for t in range(min(6, L_steps)):
                issue_xpose(t)
            for t in range(KG, min(2 * KG, L_steps)):
                issue_hgather(t)

            gru_ps = {}
            gru_mid = {}

            def gru_A(t, h):
                """MMs + sigma(r,z) + t,v for (step t, half h)."""
                if h == 0 and t not in gru_ps:
                    rz_ps = PS.tile([128, 512], f32, tag="rz", bufs=3,
                                    name=f"rz_{t}")
                    nxh_ps = PS.tile([128, 512], f32, tag="nxh", bufs=3,
                                     name=f"nxh_{t}")
                    gru_ps[t] = (rz_ps, nxh_ps)
                    xT_t = xT_ring[:, t % XRING, :]
                    nc.tensor.matmul(rz_ps[:, 0:256], wiT[:, 0, :], xT_t,
                                     start=True, stop=False, skip_group_check=True)
                    nc.tensor.matmul(rz_ps[:, 256:512], wiT[:, 1, :], xT_t,
                                     start=True, stop=False, skip_group_check=True)
                    nc.tensor.matmul(nxh_ps[:, 0:256], wiT[:, 2, :], xT_t,
                                     start=True, stop=False, skip_group_check=True)
                    if nonzero_bias:
                        nc.tensor.matmul(rz_ps[:, 0:256], grz_bias_bf[:, 0:1], bsel[:],
                                         start=False, stop=False, skip_group_check=True)
                        nc.tensor.matmul(rz_ps[:, 256:512], grz_bias_bf[:, 1:2], bsel[:],
                                         start=False, stop=False, skip_group_check=True)
                        nc.tensor.matmul(nxh_ps[:, 256:512], gbh_bf[:, 2:3], bsel[:],
                                         start=True, stop=False, skip_group_check=True)
                rz_ps, nxh_ps = gru_ps[t]
                h_prev = zeros_bf[:] if t == 0 else hs[:, t - 1, :]
                hp = h_prev[:, h * HB:(h + 1) * HB]
                nc.tensor.matmul(rz_ps[:, h * HB:h * HB + HB], whT[:, 0, :], hp,
                                 start=False, stop=False, skip_group_check=True)
                nc.tensor.matmul(rz_ps[:, 256 + h * HB:256 + h * HB + HB],
                                 whT[:, 1, :], hp,
                                 start=False, stop=True, skip_group_check=True)
                nc.tensor.matmul(nxh_ps[:, 256 + h * HB:256 + h * HB + HB],
                                 whT[:, 2, :], hp,
                                 start=(not nonzero_bias), stop=True,
                                 skip_group_check=True)
                rz4 = rz_ps[:].rearrange("p (a b) -> p a b", b=128)
                rz_sb = WK.tile([128, 256], bf16, tag=f"rz{h}", name=f"rz_{t}_{h}")
                nc.scalar.activation(out=rz_sb[:].rearrange("p (a b) -> p a b", b=128),
                                     in_=rz4[:, h::2, :], func=AF.Sigmoid)
                t_sb = WK.tile([128, 128], bf16, tag=f"t{h}", name=f"t_{t}_{h}")
                nc.vector.tensor_tensor(
                    out=t_sb[:], in0=rz_sb[:, 0:128],
                    in1=nxh_ps[:, 256 + h * HB:256 + h * HB + HB], op=OP.mult)
                v_sb = WK.tile([128, 128], bf16, tag=f"v{h}", name=f"v_{t}_{h}")
                nc.vector.tensor_tensor(
                    out=v_sb[:], in0=t_sb[:],
                    in1=nxh_ps[:, h * HB:h * HB + HB], op=OP.add)
                # p1 = z * h_prev (off the critical chain)
                p1_sb = WK.tile([128, 128], bf16, tag=f"p1{h}", name=f"p1_{t}_{h}")
                nc.vector.tensor_tensor(out=p1_sb[:], in0=rz_sb[:, 128:256], in1=hp,
                                        op=OP.mult)
                gru_mid[(t, h)] = (rz_sb, v_sb, p1_sb)

            def gru_B(t, h):
                """tanh + 2-op state update: h' = p1 - (z-1)*n."""
                rz_sb, v_sb, p1_sb = gru_mid.pop((t, h))
                n_sb = WK.tile([128, 128], bf16, tag=f"n{h}", name=f"n_{t}_{h}")
                nc.scalar.activation(out=n_sb[:], in_=v_sb[:], func=AF.Tanh,
                                     bias=gbi_sb[:, 2:3] if nonzero_bias else 0.0)
                g_sb = WK.tile([128, 128], bf16, tag=f"g{h}", name=f"g_{t}_{h}")
                nc.vector.scalar_tensor_tensor(out=g_sb[:], in0=rz_sb[:, 128:256],
                                               scalar=1.0, in1=n_sb[:],
                                               op0=OP.subtract, op1=OP.mult)
                nc.vector.tensor_tensor(out=hs[:, t, h * HB:(h + 1) * HB],
                                        in0=p1_sb[:], in1=g_sb[:], op=OP.subtract)

            def gru_prefetch(t):
                if t + 2 * KG < L_steps:
                    issue_hgather(t + 2 * KG)
                if (t + KG) % KG == 0 and t + KG < L_steps:
                    issue_cast((t + KG) // KG)
                if t + 6 < L_steps:
                    issue_xpose(t + 6)

            # staggered emission: h1 runs half a step behind h0
            gru_A(0, 0)
            for t in range(L_steps):
                gru_prefetch(t)
                if t > 0:
                    gru_B(t - 1, 1)
                gru_B(t, 0)
                gru_A(t, 1)
                if t + 1 < L_steps:
                    gru_A(t + 1, 0)
            gru_B(L_steps - 1, 1)

            # ======== attention scores (PE; scores^T layout) ========
            sc_sb = P.tile([128, ntc, B], f32)
            for ci, (c0, rows) in enumerate(tchunks):
                sc_ps = PS.tile([128, 256], f32, tag="sc")
                for b in range(B):
                    nc.tensor.matmul(sc_ps[:rows, b:b + 1], hs[:, c0:c0 + rows, b:b + 1],
                                     tgt_bf[:, b:b + 1],
                                     start=True, stop=True, skip_group_check=True)
                nc.vector.tensor_copy(out=sc_sb[:rows, ci, :], in_=sc_ps[:rows, :])

            att_work = []
            for h in range(2):
                scb = P.tile([128, L_steps], f32, tag=f"scb{h}")
                for ci, (c0, rows) in enumerate(tchunks):
                    pst5 = PS.tile([128, 256], f32, tag="sc")
                    nc.tensor.transpose(pst5[:128, :rows],
                                        sc_sb[:rows, ci, h * HB:(h + 1) * HB],
                                        ident_f32[:rows, :rows])
                    nc.vector.tensor_copy(out=scb[:, c0:c0 + rows], in_=pst5[:128, :rows])
                att_work.append(scb)

            # ======== mask + softmax (b-major) -> attnT (t-major, bf16) ========
            attnT_sb = P.tile([128, ntc, B], bf16)
            for h in range(2):
                scb = att_work[h]
                m01 = WK.tile([128, L_steps], f32, tag="m01")
                nc.vector.tensor_scalar(out=m01[:], in0=seq_sb[:, h, :], scalar1=0,
                                        scalar2=None, op0=OP.is_gt)
                sm = WK.tile([128, L_steps], f32, tag="sm")
                nc.vector.scalar_tensor_tensor(out=sm[:], in0=scb[:], scalar=1e9,
                                               in1=m01[:], op0=OP.add, op1=OP.mult)
                rmax = WK.tile([128, 1], f32, tag="rmax")
                nc.vector.tensor_reduce(out=rmax[:], in_=sm[:],
                                        axis=mybir.AxisListType.X, op=OP.max, negate=True)
                ex = WK.tile([128, L_steps], f32, tag="ex")
                nc.scalar.activation(out=ex[:], in_=sm[:], func=AF.Exp, bias=rmax[:])
                rsum = WK.tile([128, 1], f32, tag="rsum")
                nc.vector.tensor_reduce(out=rsum[:], in_=ex[:],
                                        axis=mybir.AxisListType.X, op=OP.add)
                rinv = WK.tile([128, 1], f32, tag="rinv")
                nc.vector.reciprocal(out=rinv[:], in_=rsum[:])
                attn_b = WK.tile([128, L_steps], bf16, tag="attnb")
                nc.vector.tensor_scalar(out=attn_b[:], in0=ex[:], scalar1=rinv[:],
                                        scalar2=None, op0=OP.mult)
                for ci, (c0, rows) in enumerate(tchunks):
                    pst6 = PS.tile([128, 256], bf16, tag="sc")
                    nc.tensor.transpose(pst6[:rows, :128], attn_b[:, c0:c0 + rows],
                                        ident_bf[:])
                    nc.vector.tensor_copy(out=attnT_sb[:rows, ci, h * HB:(h + 1) * HB],
                                          in_=pst6[:rows, :128])
            for ci, (c0, rows) in enumerate(tchunks):
                nc.sync.dma_start(out=attnT_dram[c0:c0 + rows, :],
                                  in_=attnT_sb[:rows, ci, :])

            # ======== AUGRU scan ========
            def issue_abcast(t0):
                kw = min(ABC, L_steps - t0)
                src = attnT_dram[t0:t0 + kw, :]
                src_b = bass.AP(tensor=src.tensor, offset=src.offset,
                                ap=[[0, 128]] + list(src.ap))
                nc.gpsimd.dma_start(out=a_ring[:, t0 % ARING:t0 % ARING + kw, :],
                                    in_=src_b)

            for t0 in range(0, min(ARING, L_steps), ABC):
                issue_abcast(t0)

            au_ps = {}
            au_mid = {}

            def au_A(t, h):
                if h == 0 and t not in au_ps:
                    ru_ps = PS.tile([128, 512], f32, tag="rz", bufs=3,
                                    name=f"ru_{t}")
                    n_ps = PS.tile([128, 256], f32, tag="nxh", bufs=3,
                                   name=f"n_{t}")
                    au_ps[t] = (ru_ps, n_ps)
                    x_t = hs[:, t, :]
                    nc.tensor.matmul(ru_ps[:, 0:256], auT[:, 0, :], x_t,
                                     start=True, stop=False, skip_group_check=True)
                    nc.tensor.matmul(ru_ps[:, 256:512], auT[:, 2, :], x_t,
                                     start=True, stop=False, skip_group_check=True)
                    nc.tensor.matmul(n_ps[:, 0:256], auT[:, 4, :], x_t,
                                     start=True, stop=False, skip_group_check=True)
                    if nonzero_bias:
                        nc.tensor.matmul(ru_ps[:, 0:256], aub_bf[:, 0:1], bsel[:],
                                         start=False, stop=False, skip_group_check=True)
                        nc.tensor.matmul(ru_ps[:, 256:512], aub_bf[:, 1:2], bsel[:],
                                         start=False, stop=False, skip_group_check=True)
                ru_ps, n_ps = au_ps[t]
                h_prev = zeros_bf[:] if t == 0 else ah[:, (t - 1) % 2, :]
                hp = h_prev[:, h * HB:(h + 1) * HB]
                nc.tensor.matmul(ru_ps[:, h * HB:h * HB + HB], auT[:, 1, :], hp,
                                 start=False, stop=False, skip_group_check=True)
                nc.tensor.matmul(ru_ps[:, 256 + h * HB:256 + h * HB + HB],
                                 auT[:, 3, :], hp,
                                 start=False, stop=True, skip_group_check=True)
                ru4 = ru_ps[:].rearrange("p (a b) -> p a b", b=128)
                ru_sb = WK.tile([128, 256], bf16, tag=f"ru{h}", name=f"ru_{t}_{h}")
                nc.scalar.activation(out=ru_sb[:].rearrange("p (a b) -> p a b", b=128),
                                     in_=ru4[:, h::2, :], func=AF.Sigmoid)
                rh_sb = WK.tile([128, 128], bf16, tag=f"rh{h}", name=f"rh_{t}_{h}")
                nc.vector.tensor_tensor(out=rh_sb[:], in0=ru_sb[:, 0:128], in1=hp,
                                        op=OP.mult)
                nc.tensor.matmul(n_ps[:, h * HB:h * HB + HB], auT[:, 5, :], rh_sb[:],
                                 start=False, stop=True, skip_group_check=True)
                # w = u * a_t ; q1 = (w-1)*h   (both off the critical chain)
                w_sb = WK.tile([128, 128], bf16, tag=f"w{h}", name=f"w_{t}_{h}")
                nc.vector.tensor_tensor(out=w_sb[:], in0=ru_sb[:, 128:256],
                                        in1=a_ring[:, t % ARING, h * HB:(h + 1) * HB],
                                        op=OP.mult)
                q1_sb = WK.tile([128, 128], bf16, tag=f"q1{h}", name=f"q1_{t}_{h}")
                nc.vector.scalar_tensor_tensor(out=q1_sb[:], in0=w_sb[:], scalar=1.0,
                                               in1=hp, op0=OP.subtract, op1=OP.mult)
                au_mid[(t, h)] = (w_sb, q1_sb)

            def au_B(t, h):
                w_sb, q1_sb = au_mid.pop((t, h))
                ru_ps, n_ps = au_ps[t]
                n2_sb = WK.tile([128, 128], bf16, tag=f"n2{h}", name=f"n2_{t}_{h}")
                nc.scalar.activation(out=n2_sb[:], in_=n_ps[:, h * HB:h * HB + HB],
                                     func=AF.Tanh,
                                     bias=aub_sb[:, 2:3] if nonzero_bias else 0.0)
                m1_sb = WK.tile([128, 128], bf16, tag=f"m1{h}", name=f"m1_{t}_{h}")
                nc.vector.tensor_tensor(out=m1_sb[:], in0=w_sb[:], in1=n2_sb[:],
                                        op=OP.mult)
                nc.vector.tensor_tensor(out=ah[:, t % 2, h * HB:(h + 1) * HB],
                                        in0=m1_sb[:], in1=q1_sb[:], op=OP.subtract)

            au_A(0, 0)
            for t in range(L_steps):
                tf = t + ARING // 2
                if tf < L_steps and tf % ABC == 0:
                    issue_abcast(tf)
                if t > 0:
                    au_B(t - 1, 1)
                au_B(t, 0)
                au_A(t, 1)
                if t + 1 < L_steps:
                    au_A(t + 1, 0)
            au_B(L_steps - 1, 1)

            evolved = ah[:, (L_steps - 1) % 2, :]
            grp_rhs = dict(grp_sb)
            grp_rhs["ev"] = evolved

            # ======== MLP head ========
            h1_sb = P.tile([128, 2, B], bf16)
            for m in range(2):
                h1_ps = PS.tile([128, 256], f32, tag="nxh", bufs=3)
                for gi, (gname, off, w) in enumerate(GROUPS):
                    nc.tensor.matmul(h1_ps[:, :], w1T[gname][:w, m, :],
                                     grp_rhs[gname][:] if gname == "ev" else grp_rhs[gname][:w, :],
                                     start=(gi == 0), stop=(gi == len(GROUPS) - 1),
                                     skip_group_check=True)
                nc.scalar.activation(out=h1_sb[:, m, :], in_=h1_ps[:, :], func=AF.Relu,
                                     bias=b1_sb[:, m:m + 1])
            h2_ps = PS.tile([128, 256], f32, tag="rz", bufs=3)
            nc.tensor.matmul(h2_ps[:, :], w2T[:, 0, :], h1_sb[:, 0, :],
                             start=True, stop=False, skip_group_check=True)
            nc.tensor.matmul(h2_ps[:, :], w2T[:, 1, :], h1_sb[:, 1, :],
                             start=False, stop=True, skip_group_check=True)
            h2_sb = P.tile([128, B], bf16)
            nc.scalar.activation(out=h2_sb[:], in_=h2_ps[:, :], func=AF.Relu,
                                 bias=b2_sb[:])
            lg_ps = PS.tile([1, 256], f32, tag="sc")
            nc.tensor.matmul(lg_ps[:, :], w3T[:], h2_sb[:],
                             start=True, stop=True, skip_group_check=True)
            out_sb = P.tile([1, B], f32)
            nc.scalar.activation(out=out_sb[:], in_=lg_ps[:, :], func=AF.Sigmoid,
                                 bias=b3_sb[0:1, :])
            nc.sync.dma_start(out=out_d.rearrange("(a b) -> a b", a=1), in_=out_sb[:])

    nc.finalize()
    return nc


_NC_CACHE = {}


def kernel(**inputs):
    from concourse import bass_utils

    inputs = {k: np.asarray(v) for k, v in inputs.items()}
    L_steps = inputs["history_seq"].shape[1]
    bias_names = ["gru_bi", "gru_bh", "au_br", "au_bu", "au_bh"]
    nonzero_bias = any(np.any(inputs[k]) for k in bias_names)

    key = (L_steps, nonzero_bias)
    if key not in _NC_CACHE:
        _NC_CACHE[key] = build_dien(L_steps, nonzero_bias)
    nc = _NC_CACHE[key]

    per_b = ["user_id", "item_id", "item_category", "item_dur_bkt",
             "history_seq", "user_dense", "item_dense"]
    shared = {k: np.ascontiguousarray(v) for k, v in inputs.items() if k not in per_b}
    in_maps = []
    for c in range(NCORES):
        m = dict(shared)
        for k in per_b:
            m[k] = np.ascontiguousarray(inputs[k][c * B:(c + 1) * B])
        in_maps.append(m)

    res = bass_utils.run_bass_kernel_spmd(nc, in_maps, core_ids=list(range(NCORES)))
    out = np.concatenate([res.results[c]["out"] for c in range(NCORES)])
    return out.astype(np.float32)


if __name__ == "__main__":
    import os
    import importlib.util
    spec = importlib.util.spec_from_file_location("reference", "/root/problem/reference.py")
    ref = importlib.util.module_from_spec(spec)
    spec.loader.exec_module(ref)
    ins = {k: np.asarray(v) for k, v in ref.setup_inputs().items()}
    Lt = int(os.environ.get("DIEN_L", "8"))
    if Lt < L:
        ins["history_seq"] = np.ascontiguousarray(ins["history_seq"][:, :Lt])
    import jax.numpy as jnp
    exp = np.asarray(ref.reference(**{k: jnp.asarray(v) for k, v in ins.items()}))
    got = kernel(**ins)
    err = np.abs(got - exp)
    rel = err / np.maximum(np.abs(exp), 1e-6)
    print(f"L={Lt} max_abs={err.max():.3e} max_rel={rel.max():.3e} mean_rel={rel.mean():.3e}")
